# revision 1
# baseline (speedup 1.0000x reference)
"""Trainium2 Bass kernel: batched multi-head attention, data-parallel over batch.

Full inputs: query/key/value (8, 12, 512, 256) fp32; 8 heads x head_dim 32;
softmax over the 512 keys for each (batch, step, head, query-node).

Sharding: batch (=8) across the 8 NeuronCores; each core computes its
(12, 512, 256) slice independently. No collectives.

Per-core algorithm, per step t (12 steps), per head-half (4 heads):
  - Load Q,K natural as bf16 (cast DMA), xbar-DMA-transpose 128x128 blocks to
    get Q^T,K^T with head_dim on partitions.
  - scores_T[k,q] = K^T-chunk.T @ Q^T: contraction is head_dim=32, so two
    heads run concurrently in the PE array via row tiling (tile_position).
  - exp of the scores is split across TWO engines per half-step: ScalarE
    computes exact exp on 5 of the 8 (128,1024) score tiles; VectorE computes
    the other 3 via a custom DVE op implementing the Schraudolph bit trick:
    int16(x*C0 + C1) produces the bit pattern of bf16 2^(x*SCALE*log2e) with
    ~+-3% per-element error that averages out in the softmax (verified
    end-to-end rel err ~1e-2 < 2e-2 budget). This splits the former single
    ScalarE exp bottleneck (~220us) across ACT and DVE (~137us each).
  - out_T = [V | ones].T @ exp_T accumulated over the 4 k-chunks; the ones
    column yields the softmax denominator for free. Two heads per PSUM bank
    (partitions 0-32 / 64-96, concurrently via column tiling). V is loaded
    pre-interleaved [V_h | 1] by a strided cast-DMA (no DVE interleave pass).
  - One fused DVE copy moves the (97,1024) AV PSUM block to SBUF; PE
    transposes it back to (q, d); reciprocal + broadcast-multiply read the
    transpose PSUM directly (no SBUF compaction pass); DMA out.
  - The (QK+exp) stage of each half is emitted one half-stage AHEAD of the
    AV+output stage so the PE always has the next half's scores ready for
    ACT/DVE (keeps both exp engines gap-free at t boundaries).
"""

import numpy as np

import concourse.bass as bass
import concourse.mybir as mybir
import concourse.tile as tile
from concourse import bacc
from concourse.masks import make_identity
from concourse.bass_utils import run_bass_kernel_spmd

B, T, N, D = 8, 12, 512, 256
H, HD = 8, 32
SCALE = 1.0 / float(np.sqrt(HD))
NCORES = 8

F32 = mybir.dt.float32
BF16 = mybir.dt.bfloat16

QK_DTYPE = BF16   # Q/K operand dtype (bf16 enables xbar DMA transpose)
EXP_DTYPE = BF16  # sbuf dtype of exp(scores) == AV rhs operand dtype
V_DTYPE = BF16    # sbuf dtype of V (+ones)  == AV lhsT operand dtype
import os as _os0
PIPE_STAGES = int(_os0.environ.get("K_PIPE", "2"))  # half-stage lookahead

# ---- custom DVE op: Schraudolph exp -> bf16 bit pattern --------------------
# out_bits(int16) = round(x*C0 + C1); viewed as bf16 this is
# 2^(x*SCALE*log2e + centering) = exp(x*SCALE) * (1 + eps), |eps| <~ 3%.
# C0 maps raw scores to 128ths-of-exponent units; C1 = (127 - c)*128 with
# c = E_f[log2(1+f) - f] = 0.057305 centering the tent-shaped log error.
SCHR_C0 = 128.0 * float(np.log2(np.e)) * SCALE
SCHR_C1 = (127.0 - 0.057304959111036) * 128.0
# score tiles (kc, pr) routed to the DVE each half-step; rest go to ScalarE.
# Priority order spreads approximated tiles across key-chunks first.
_DVE_ORDER = [(0, 1), (1, 1), (2, 1), (0, 0), (1, 0), (2, 0), (3, 1), (3, 0)]
import os as _os
N_DVE = int(_os.environ.get("K_N_DVE", "4"))   # tiles/half routed to DVE
AVS_ENGINE = _os.environ.get("K_AVS", "act")   # 'act' | 'dve' | 'alt'
if _os.environ.get("K_DVE_ALT", "0") == "1" and N_DVE == 4:
    # strict engine alternation on the score-tile ring: emission order is
    # kc-major (kc,0),(kc,1), so routing all pr=0 tiles to DVE yields
    # D,A,D,A,... and neither exp queue gates PSUM buffer recycling
    DVE_TILES = frozenset({(0, 0), (1, 0), (2, 0), (3, 0)})
else:
    DVE_TILES = frozenset(_DVE_ORDER[:N_DVE])


def _register_schraudolph_op():
    import concourse.dve_ops as dve_ops
    from concourse.dve_spec import Spec, Src0, C0, C1, lower, _has_src1
    from concourse.dve_uop import DveOpSpec

    name = "SCHRAUDOLPH_EXP_ANT"
    for o in dve_ops.OPS:
        if o.name == name:
            return o
    spec = Spec(
        body=Src0 * C0 + C1,
        reference=lambda in0, in1, s0, s1, imm2: (
            in0.astype(np.float32) * np.float32(s0) + np.float32(s1)
        ),
    )
    row = dve_ops._CUSTOM_DVE_ROW_BASE + len(dve_ops.OPS)
    dve_ops._SUB_OPCODE_FOR_NAME[name] = row
    shas = {}
    for ver in ("v3", "v4"):
        s = DveOpSpec(
            name=name, opcode=row, uops=lower(spec, ver=ver),
            rd1_en=_has_src1(spec),
        )
        shas[ver] = s.sha(ver)
    op = dve_ops.DveOp(name, spec, subdim=False, uops_sha=shas)
    dve_ops.OPS.append(op)
    dve_ops.CUSTOM_DVE_SPECS[name] = spec
    return op


SCHR_OP = _register_schraudolph_op()


def _attention_body(tc, o_ext, q_ext, k_ext, v_ext, reps=1):
    nc = tc.nc
    Exp = mybir.ActivationFunctionType.Exp

    with (
        tc.tile_pool(name="const", bufs=1) as const_pool,
        tc.tile_pool(name="qk", bufs=3) as qk_pool,
        tc.tile_pool(name="vsb", bufs=4) as v_pool,
        tc.tile_pool(name="exp", bufs=24) as exp_pool,
        tc.tile_pool(name="avs", bufs=3) as avs_pool,
        tc.tile_pool(name="rec", bufs=3) as rec_pool,
        tc.tile_pool(name="fout", bufs=4) as fout_pool,
        tc.tile_pool(name="scorep", bufs=2, space="PSUM") as scores_pool,
        tc.tile_pool(name="avp", bufs=1, space="PSUM") as av_pool,
        tc.tile_pool(name="trp", bufs=1, space="PSUM") as tr_pool,
    ):
        ident = const_pool.tile([128, 128], F32)
        warm = const_pool.tile([128, 1], F32)
        vset = {}
        for s in range(2):
            for half in range(2):
                vsb = v_pool.tile([128, 4, 132], V_DTYPE, tag="vsb")
                vset[s, half] = vsb
        qkT = {}

        def emit_preamble():
            # everything here is off the first-exp critical path; emitted
            # after t=0's Q/K loads so the DMA queues start those first
            make_identity(nc, ident[:])
            nc.scalar.activation(warm[:], ident[:, 0:1], Exp)  # exp table load
            # persistent V (+ones) tiles, double-buffered by t parity; the
            # ones columns are written once here and never overwritten.
            for s in range(2):
                for half in range(2):
                    nc.gpsimd.memset(vset[s, half][:], 1.0)

        def emit_qk_loads(j, t):
            # natural-layout loads (cast f32 -> bf16 during DMA, SWDGE),
            # split per head-half so half-0 transposes can start while the
            # half-1 columns are still loading
            qnat = qk_pool.tile([128, 4, 256], QK_DTYPE, tag="qnat")
            knat = qk_pool.tile([128, 4, 256], QK_DTYPE, tag="knat")
            for half in range(2):
                cols = slice(half * 128, (half + 1) * 128)
                nc.gpsimd.dma_start(
                    out=qnat[:, :, cols],
                    in_=q_ext[t].rearrange("(c p) d -> p c d", p=128)[
                        :, :, cols
                    ],
                )
                nc.gpsimd.dma_start(
                    out=knat[:, :, cols],
                    in_=k_ext[t].rearrange("(c p) d -> p c d", p=128)[
                        :, :, cols
                    ],
                )

            # transposes via xbar DMA: qT[d, half, q] (head_dim on partitions)
            qT = qk_pool.tile([128, 2, 512], QK_DTYPE, tag="qT")
            kT = qk_pool.tile([128, 2, 512], QK_DTYPE, tag="kT")
            for half in range(2):
                for c in range(4):
                    nc.sync.dma_start_transpose(
                        out=qT[:, half, c * 128:(c + 1) * 128],
                        in_=qnat[:, c, half * 128:(half + 1) * 128],
                    )
                    nc.sync.dma_start_transpose(
                        out=kT[:, half, c * 128:(c + 1) * 128],
                        in_=knat[:, c, half * 128:(half + 1) * 128],
                    )
            qkT[j] = (qT, kT)

        def emit_v_load(t):
            # V: one natural cast-load (f32 -> bf16 DMA), then a cheap bf16
            # DVE interleave into the persistent [V_h | 1] tiles (the ones
            # columns at offset 32 of each 33-group survive the memset)
            vtmp = qk_pool.tile([128, 4, 256], V_DTYPE, tag="vtmp")
            nc.gpsimd.dma_start(
                out=vtmp[:], in_=v_ext[t].rearrange("(kc p) d -> p kc d", p=128)
            )
            for half in range(2):
                nc.vector.tensor_copy(
                    vset[t % 2, half][:].rearrange(
                        "p kc (h x) -> p kc h x", x=33
                    )[:, :, :, 0:32],
                    vtmp[:, :, half * 128:(half + 1) * 128].rearrange(
                        "p kc (h d) -> p kc h d", d=32
                    ),
                )

        def emit_qk_exp(j, t, half):
            qT, kT = qkT[j]
            exps = {}
            for kc in range(4):
                for pr in range(2):
                    scores = scores_pool.tile([128, 1024], F32)  # 2 banks
                    for sub in range(2):
                        r = pr * 64 + sub * 32
                        nc.tensor.matmul(
                            scores[:, sub * 512:(sub + 1) * 512],
                            lhsT=kT[r:r + 32, half, kc * 128:(kc + 1) * 128],
                            rhs=qT[r:r + 32, half, :],
                            start=True, stop=True,
                            tile_position=(r, 0),
                        )
                    exp_t = exp_pool.tile([128, 1024], EXP_DTYPE, tag="exp")
                    if (kc, pr) in DVE_TILES:
                        nc.vector._custom_dve(
                            SCHR_OP,
                            out=exp_t[:].bitcast(mybir.dt.int16),
                            in0=scores[:],
                            s0=SCHR_C0, s1=SCHR_C1,
                        )
                    else:
                        nc.scalar.activation(
                            exp_t[:], scores[:], Exp, scale=SCALE
                        )
                    exps[kc, pr] = exp_t
            return exps

        def emit_av_out(t, half, exps):
            # AV with ones-column denominators; all 4 heads of the half in
            # one 2-bank PSUM tile so the SBUF spill is a single DVE copy
            av = av_pool.tile([128, 1024], F32)
            for b in range(2):
                for j in range(2):
                    i = b * 2 + j
                    for kc in range(4):
                        nc.tensor.matmul(
                            av[64 * j:64 * j + 33, b * 512:(b + 1) * 512],
                            lhsT=vset[t % 2, half][:, kc, i * 33:(i + 1) * 33],
                            rhs=exps[kc, i // 2][
                                :, (i % 2) * 512:(i % 2 + 1) * 512
                            ],
                            start=(kc == 0), stop=(kc == 3),
                        )

            # PSUM -> SBUF spill: on ScalarE it lands right after the next
            # half's exp ops in the ACT queue, so the single-buffered av tile
            # frees exactly when the next AV group needs it (no PE<->DVE
            # ping-pong); 'alt' splits the cost across both engines.
            avs = avs_pool.tile([128, 1024], F32, tag="avs")
            eng = AVS_ENGINE
            if eng == "alt":
                eng = "act" if (t * 2 + half) % 2 == 0 else "dve"
            if eng == "act":
                nc.scalar.copy(avs[0:97, :], av[0:97, :])
            else:
                nc.vector.tensor_copy(avs[0:97, :], av[0:97, :])

            # transpose back to (q, d) orientation, 128x128 blocks on the PE
            trp = tr_pool.tile([128, 1024], F32)
            for b in range(2):
                for c in range(4):
                    nc.tensor.transpose(
                        trp[:, b * 512 + c * 128:b * 512 + (c + 1) * 128],
                        avs[:, b * 512 + c * 128:b * 512 + (c + 1) * 128],
                        ident[:],
                    )

            # denominators sit at offset 32 of each 64-col group of trp;
            # reciprocal + broadcast-multiply read the transpose PSUM direct
            rec = rec_pool.tile([128, 16], F32, tag="rec")
            nc.vector.reciprocal(
                rec[:].rearrange("p (x e) -> p x e", e=1),
                trp[:].rearrange("p (x e) -> p x e", e=64)[:, :, 32:33],
            )

            fout = fout_pool.tile([128, 512], F32, tag="fout")
            for b in range(2):
                in0 = trp[:, b * 512:(b + 1) * 512].rearrange(
                    "p (c j e) -> p c j e", j=2, e=64
                )[:, :, :, 0:32]
                outap = fout[:].rearrange(
                    "p (c x d) -> p c x d", x=4, d=32
                )[:, :, b * 2:b * 2 + 2, :]
                recap = rec[:].rearrange(
                    "p (bb c j) -> p bb c j", bb=2, c=4
                )[:, b].unsqueeze(3).broadcast_to((128, 4, 2, 32))
                nc.vector.tensor_mul(outap, in0, recap)

            nc.sync.dma_start(
                out=o_ext[t].rearrange("(c p) d -> p c d", p=128)[
                    :, :, half * 128:(half + 1) * 128
                ],
                in_=fout[:].rearrange("p (c d) -> p c d", d=128),
            )

        # Q/K loads+transposes are emitted one t-step AHEAD so the xbar DMA
        # latency never stalls the first QK matmul of a step.
        steps = [t for _ in range(reps) for t in range(T)]
        pending = []
        first = True
        for j in range(len(steps)):
            t = steps[j]
            for half in range(2):
                if half == 0:
                    if first:
                        emit_qk_loads(j, t)
                        emit_preamble()
                        first = False
                    if j + 1 < len(steps):
                        emit_qk_loads(j + 1, steps[j + 1])
                    emit_v_load(t)
                pending.append((t, half, emit_qk_exp(j, t, half)))
                if len(pending) > PIPE_STAGES:
                    emit_av_out(*pending.pop(0))
        for item in pending:
            emit_av_out(*item)


def build_program(enable_asserts=False, reps=1):
    nc = bacc.Bacc(
        "TRN2",
        target_bir_lowering=False,
        debug=False,
        enable_asserts=enable_asserts,
        num_devices=NCORES,
    )
    q_ext = nc.dram_tensor("q", [T, N, D], F32, kind="ExternalInput").ap()
    k_ext = nc.dram_tensor("k", [T, N, D], F32, kind="ExternalInput").ap()
    v_ext = nc.dram_tensor("v", [T, N, D], F32, kind="ExternalInput").ap()
    o_ext = nc.dram_tensor("out", [T, N, D], F32, kind="ExternalOutput").ap()
    with tile.TileContext(nc) as tc:
        _attention_body(tc, o_ext, q_ext, k_ext, v_ext, reps=reps)
    nc.compile()
    return nc


_NC_CACHE = None


def _get_nc():
    global _NC_CACHE
    if _NC_CACHE is None:
        _NC_CACHE = build_program()
    return _NC_CACHE


def run(query, key, value, trace=False):
    """Run on the 8 NeuronCores; returns (out, exec_time_ns_or_None)."""
    nc = _get_nc()
    in_maps = [
        {
            "q": np.ascontiguousarray(np.asarray(query[i], dtype=np.float32)),
            "k": np.ascontiguousarray(np.asarray(key[i], dtype=np.float32)),
            "v": np.ascontiguousarray(np.asarray(value[i], dtype=np.float32)),
        }
        for i in range(NCORES)
    ]
    res = run_bass_kernel_spmd(nc, in_maps, core_ids=list(range(NCORES)),
                               trace=trace)
    out = np.stack([np.asarray(res.results[i]["out"]) for i in range(NCORES)])
    return out, res.exec_time_ns


def kernel(query, key, value):
    out, _ = run(query, key, value, trace=False)
    return out



# revision 2
# speedup vs baseline: 1.5231x; 1.5231x over previous
"""Trainium2 Bass kernel: batched multi-head attention, data-parallel over batch.

Full inputs: query/key/value (8, 12, 512, 256) fp32; 8 heads x head_dim 32;
softmax over the 512 keys for each (batch, step, head, query-node).

Sharding: batch (=8) across the 8 NeuronCores; each core computes its
(12, 512, 256) slice independently. No collectives.

v2 design (vs baseline): the kernel is paced by PSUM evacuation -- every
score element must leave PSUM f32 through ACT (153.6G elem/s) or DVE
(122.9G elem/s, single PSUM port), a ~100us/core floor. Everything else is
arranged so both engines stay saturated and the PE never blocks them:

  - QK: per key-chunk, 4 matmuls (one per head) on 4 distinct PE row bands
    (tile_position=(32h,0)) into 4 distinct PSUM banks -> concurrent on the
    16x 32x32-subarray PE (measured ~3x on TRN2).
  - Score ring: 3 PSUM tiles [128,1024] (6 banks), freed by dropping the
    PE-transpose PSUM tile, so QK runs ahead of exp instead of gating it.
  - exp: split ACT (exact, table exp) / DVE (Schraudolph bit-trick custom
    op) per tile, ratio tunable (K_ACT_EXP of 8 tiles per half).
  - AV: V-stationary with ones-column denominators; matmuls ordered
    kc -> (j inner) so the two 33-row head-pairs live in disjoint PE
    column-group pairs and overlap 2-way.
  - Output: AV PSUM -> SBUF fp16 spill, then xbar DMA transpose (8 128x128
    blocks) replaces the PE transpose; one DVE reciprocal + one DVE
    broadcast-multiply normalize in [q, d] orientation; SWDGE cast-DMA
    (fp16 -> f32) writes DRAM.  (K_OUT=direct: exp-stationary AV writes
    [q, d] directly, no spill/transpose, at 64 LDWEIGHTS/half on the PE.)
  - V interleave ([V_h | 1] tiles) on GPSIMD, off the two critical engines.
"""

import os
import numpy as np

import concourse.bass as bass
import concourse.mybir as mybir
import concourse.tile as tile
from concourse import bacc
from concourse.bass_utils import run_bass_kernel_spmd

B, T, N, D = 8, 12, 512, 256
H, HD = 8, 32
SCALE = 1.0 / float(np.sqrt(HD))
NCORES = 8

F32 = mybir.dt.float32
F16 = mybir.dt.float16
BF16 = mybir.dt.bfloat16

QK_DTYPE = BF16   # Q/K operand dtype (bf16 enables xbar DMA transpose)
EXP_DTYPE = BF16  # sbuf dtype of exp(scores) == AV operand dtype
V_DTYPE = BF16    # sbuf dtype of V (+ones) == AV operand dtype

OUT_PATH = os.environ.get("K_OUT", "pe2")   # 'xbar'|'direct'|'pe'|'pe2'
N_ACT_EXP = 4
SPILL_ENGINE = "act"
VINT_ENGINE = "gpsimd"
PIPE_STAGES = 2
OUT_F32 = True
N_RING = 0
SKIP_AV = False
QK_ONLY = False
QK16 = True
AV_KC_INNER = False

# ---- custom DVE op: Schraudolph exp -> bf16 bit pattern --------------------
# out_bits(int16) = round(x*C0 + C1); viewed as bf16 this is
# 2^(x*SCALE*log2e + centering) = exp(x*SCALE) * (1 + eps), |eps| <~ 3%.
SCHR_C0 = 128.0 * float(np.log2(np.e)) * SCALE
SCHR_C1 = (127.0 - 0.057304959111036) * 128.0


def _register_schraudolph_op():
    import concourse.dve_ops as dve_ops
    from concourse.dve_spec import Spec, Src0, C0, C1, lower, _has_src1
    from concourse.dve_uop import DveOpSpec

    name = "SCHRAUDOLPH_EXP_ANT"
    for o in dve_ops.OPS:
        if o.name == name:
            return o
    spec = Spec(
        body=Src0 * C0 + C1,
        reference=lambda in0, in1, s0, s1, imm2: (
            in0.astype(np.float32) * np.float32(s0) + np.float32(s1)
        ),
    )
    row = dve_ops._CUSTOM_DVE_ROW_BASE + len(dve_ops.OPS)
    dve_ops._SUB_OPCODE_FOR_NAME[name] = row
    shas = {}
    for ver in ("v3", "v4"):
        s = DveOpSpec(
            name=name, opcode=row, uops=lower(spec, ver=ver),
            rd1_en=_has_src1(spec),
        )
        shas[ver] = s.sha(ver)
    op = dve_ops.DveOp(name, spec, subdim=False, uops_sha=shas)
    dve_ops.OPS.append(op)
    dve_ops.CUSTOM_DVE_SPECS[name] = spec
    return op


SCHR_OP = _register_schraudolph_op()


def _act_tile_pattern(n_act):
    """Which of the 8 (kc, pr) score tiles per half go to ACT (True) vs DVE,
    spread evenly through the emission order so both engines stay busy."""
    pat = []
    acc = 0
    for i in range(8):
        nxt = (i + 1) * n_act // 8
        pat.append(nxt > acc)
        acc = nxt
    return pat


ACT_PAT = _act_tile_pattern(N_ACT_EXP)


def _emit_qk_mms(nc, scores, kT, qT, half, kc, pr):
    """QK matmuls for one (kc, head-pair) score tile.

    QK16: 8 matmuls of [K=32, M=32, N=512] at tile_position (32h, 32m) --
    4x4 PE tiling (with the other pr tile) so all 16 subarrays of one kc
    run concurrently.  Else: 2 full-M matmuls (128 out rows each).
    """
    for sub in range(2):
        h = pr * 2 + sub
        r = 32 * h
        if QK16:
            for m in range(4):
                nc.tensor.matmul(
                    scores[32 * m:32 * m + 32, sub * 512:(sub + 1) * 512],
                    lhsT=kT[r:r + 32, half,
                            kc * 128 + 32 * m:kc * 128 + 32 * m + 32],
                    rhs=qT[r:r + 32, half, :],
                    start=True, stop=True,
                    tile_position=(r, 32 * m),
                )
        else:
            nc.tensor.matmul(
                scores[:, sub * 512:(sub + 1) * 512],
                lhsT=kT[r:r + 32, half, kc * 128:(kc + 1) * 128],
                rhs=qT[r:r + 32, half, :],
                start=True, stop=True,
                tile_position=(r, 0),
            )


def _attention_body(tc, o_ext, q_ext, k_ext, v_ext, reps=1):
    nc = tc.nc
    Exp = mybir.ActivationFunctionType.Exp
    direct = OUT_PATH == "direct"
    pe_out = OUT_PATH == "pe"
    pe2_out = OUT_PATH == "pe2"
    ring = N_RING or (2 if pe_out else 3)
    fout_dt = F32 if OUT_F32 else F16

    pools = [
        tc.tile_pool(name="const", bufs=1),
        tc.tile_pool(name="qk", bufs=3),
        tc.tile_pool(name="vsb", bufs=4),
        tc.tile_pool(name="exp", bufs=24),
        tc.tile_pool(name="rec", bufs=3),
        tc.tile_pool(name="fout", bufs=4),
        tc.tile_pool(name="scorep", bufs=ring, space="PSUM"),
    ]
    import contextlib
    with contextlib.ExitStack() as stack:
        (const_pool, qk_pool, v_pool, exp_pool, rec_pool, fout_pool,
         scores_pool) = [stack.enter_context(p) for p in pools]
        if direct:
            # two tags (av2a/av2b) x bufs=1 -> 2 banks, single-buffered
            av_pool = stack.enter_context(
                tc.tile_pool(name="avp", bufs=1, space="PSUM"))
        else:
            av_pool = stack.enter_context(
                tc.tile_pool(name="avp", bufs=1, space="PSUM"))
            avs_pool = stack.enter_context(tc.tile_pool(name="avs", bufs=3))
            if pe_out:
                tr_pool = stack.enter_context(
                    tc.tile_pool(name="trp", bufs=1, space="PSUM"))
            else:
                trsb_pool = stack.enter_context(
                    tc.tile_pool(name="trsb", bufs=3))

        warm = const_pool.tile([128, 1], F32)
        warm_src = const_pool.tile([128, 1], F32)
        if pe_out or pe2_out:
            ident = const_pool.tile([128, 128], F32)
        else:
            ident = None
        vset = {}
        for s in range(2):
            for half in range(2):
                if direct:
                    vsb = v_pool.tile([128, 4, 4, 36], V_DTYPE, tag="vsb")
                else:
                    vsb = v_pool.tile([128, 4, 132], V_DTYPE, tag="vsb")
                vset[s, half] = vsb
        avs_init = []

        def emit_preamble():
            # off the first-exp critical path; emitted after t=0's Q/K loads
            nc.gpsimd.memset(warm_src[:], 0.0)
            nc.scalar.activation(warm[:], warm_src[:], Exp)  # exp table load
            if pe_out or pe2_out:
                from concourse.masks import make_identity
                make_identity(nc, ident[:])
            # persistent V (+ones) tiles; ones columns written once here.
            for s in range(2):
                for half in range(2):
                    nc.gpsimd.memset(vset[s, half][:], 1.0)
            for t_ in avs_init:
                nc.gpsimd.memset(t_[:], 0.0)

        def emit_qk_loads(j, t):
            # natural-layout loads (cast f32 -> bf16 during DMA, SWDGE),
            # split per head-half so half-0 transposes start early
            qnat = qk_pool.tile([128, 4, 256], QK_DTYPE, tag="qnat")
            knat = qk_pool.tile([128, 4, 256], QK_DTYPE, tag="knat")
            for half in range(2):
                cols = slice(half * 128, (half + 1) * 128)
                nc.gpsimd.dma_start(
                    out=qnat[:, :, cols],
                    in_=q_ext[t].rearrange("(c p) d -> p c d", p=128)[
                        :, :, cols
                    ],
                )
                nc.gpsimd.dma_start(
                    out=knat[:, :, cols],
                    in_=k_ext[t].rearrange("(c p) d -> p c d", p=128)[
                        :, :, cols
                    ],
                )

            # transposes via xbar DMA: qT[d, half, q] (head_dim on partitions)
            qT = qk_pool.tile([128, 2, 512], QK_DTYPE, tag="qT")
            kT = qk_pool.tile([128, 2, 512], QK_DTYPE, tag="kT")
            for half in range(2):
                for c in range(4):
                    nc.sync.dma_start_transpose(
                        out=qT[:, half, c * 128:(c + 1) * 128],
                        in_=qnat[:, c, half * 128:(half + 1) * 128],
                    )
                    nc.sync.dma_start_transpose(
                        out=kT[:, half, c * 128:(c + 1) * 128],
                        in_=knat[:, c, half * 128:(half + 1) * 128],
                    )
            qkT[j] = (qT, kT)

        def emit_v_load(t):
            # V natural cast-load, then interleave into the persistent
            # [V_h | 1] tiles on GPSIMD (keeps ACT/DVE free for exp)
            vtmp = qk_pool.tile([128, 4, 256], V_DTYPE, tag="vtmp")
            nc.gpsimd.dma_start(
                out=vtmp[:], in_=v_ext[t].rearrange("(kc p) d -> p kc d", p=128)
            )
            eng = nc.gpsimd if VINT_ENGINE == "gpsimd" else nc.vector
            for half in range(2):
                src = vtmp[:, :, half * 128:(half + 1) * 128].rearrange(
                    "p kc (h d) -> p kc h d", d=32
                )
                if direct:
                    dst = vset[t % 2, half][:, :, :, 0:32]
                else:
                    dst = vset[t % 2, half][:].rearrange(
                        "p kc (h x) -> p kc h x", x=33
                    )[:, :, :, 0:32]
                eng.tensor_copy(dst, src)

        def emit_qk_exp(j, t, half):
            qT, kT = qkT[j]
            exps = {}
            for kc in range(4):
                for pr in range(2):
                    scores = scores_pool.tile([128, 1024], F32)  # 2 banks
                    _emit_qk_mms(nc, scores, kT, qT, half, kc, pr)
                    exp_t = exp_pool.tile([128, 1024], EXP_DTYPE, tag="exp")
                    if QK_ONLY:
                        # probe: minimal consumer so the score ring recycles
                        nc.vector.tensor_copy(
                            exp_t[:, 0:1], scores[:, 0:1]
                        )
                    elif ACT_PAT[kc * 2 + pr]:
                        nc.scalar.activation(
                            exp_t[:], scores[:], Exp, scale=SCALE
                        )
                    else:
                        nc.vector._custom_dve(
                            SCHR_OP,
                            out=exp_t[:].bitcast(mybir.dt.int16),
                            in0=scores[:],
                            s0=SCHR_C0, s1=SCHR_C1,
                        )
                    exps[kc, pr] = exp_t
            return exps

        def emit_av_out_xbar(t, half, exps):
            # AV with ones-column denominators; kc-major, j-inner so the two
            # 33-row head pairs (col groups {0,1} vs {2,3}) overlap on the PE
            av = av_pool.tile([128, 1024], F32)
            for kc in range(4):
                for b in range(2):
                    for jj in range(2):
                        i = b * 2 + jj
                        nc.tensor.matmul(
                            av[64 * jj:64 * jj + 33, b * 512:(b + 1) * 512],
                            lhsT=vset[t % 2, half][:, kc, i * 33:(i + 1) * 33],
                            rhs=exps[kc, i // 2][
                                :, (i % 2) * 512:(i % 2 + 1) * 512
                            ],
                            start=(kc == 0), stop=(kc == 3),
                        )

            # PSUM -> SBUF fp16 spill (rows 97-127 pre-zeroed once)
            avs = avs_pool.tile([128, 1024], F16, tag="avs")
            if SPILL_ENGINE == "act":
                nc.scalar.copy(avs[0:97, :], av[0:97, :])
            elif SPILL_ENGINE == "dve":
                nc.vector.tensor_copy(avs[0:97, :], av[0:97, :])
            else:
                nc.scalar.copy(avs[0:97, 0:512], av[0:97, 0:512])
                nc.vector.tensor_copy(avs[0:97, 512:1024], av[0:97, 512:1024])

            # transpose back to (q, d) on the xbar DMA (PE stays on matmuls)
            trsb = trsb_pool.tile([128, 8, 128], F16, tag="trsb")
            for x in range(8):
                nc.sync.dma_start_transpose(
                    out=trsb[:, x, :],
                    in_=avs[:, x * 128:(x + 1) * 128],
                )

            # flat free layout of trsb is (b, c, j, s, d); the denominator of
            # head (b, j) in q-chunk c sits at s=1, d=0, i.e. every 64th
            # element at offset 32 -- same flat trick as the recip input
            rec = rec_pool.tile([128, 16], F32, tag="rec")
            nc.vector.reciprocal(
                rec[:].rearrange("p (y e) -> p y e", e=1),
                trsb[:].rearrange("p x e -> p (x e)").rearrange(
                    "p (y e) -> p y e", e=64
                )[:, :, 32:33],
            )

            # DVE tensor ops allow at most 3 free dims -> one mul per b
            fout = fout_pool.tile([128, 512], fout_dt, tag="fout")
            for b in range(2):
                nc.vector.tensor_mul(
                    fout[:].rearrange(
                        "p (c b j d) -> p b c j d", b=2, j=2, d=32
                    )[:, b],
                    trsb[:].rearrange(
                        "p (b c) (j s d) -> p b c j s d",
                        b=2, c=4, j=2, s=2, d=32,
                    )[:, b, :, :, 0, :],
                    rec[:].rearrange("p (b c j) -> p b c j", b=2, c=4, j=2)
                    [:, b].unsqueeze(3).broadcast_to((128, 4, 2, 32)),
                )

            out_ap = o_ext[t].rearrange("(c p) d -> p c d", p=128)[
                :, :, half * 128:(half + 1) * 128
            ]
            fin = fout[:].rearrange("p (c d) -> p c d", d=128)
            if OUT_F32:
                nc.sync.dma_start(out=out_ap, in_=fin)
            else:
                nc.gpsimd.dma_start(out=out_ap, in_=fin)

        def emit_av_out_pe(t, half, exps):
            # baseline output path: f32 spill + PE transpose + recip/mul
            av = av_pool.tile([128, 1024], F32)
            for kc in range(4):
                for b in range(2):
                    for jj in range(2):
                        i = b * 2 + jj
                        nc.tensor.matmul(
                            av[64 * jj:64 * jj + 33, b * 512:(b + 1) * 512],
                            lhsT=vset[t % 2, half][:, kc, i * 33:(i + 1) * 33],
                            rhs=exps[kc, i // 2][
                                :, (i % 2) * 512:(i % 2 + 1) * 512
                            ],
                            start=(kc == 0), stop=(kc == 3),
                        )

            avs = avs_pool.tile([128, 1024], F32, tag="avs")
            if SPILL_ENGINE == "act":
                nc.scalar.copy(avs[0:97, :], av[0:97, :])
            elif SPILL_ENGINE == "dve":
                nc.vector.tensor_copy(avs[0:97, :], av[0:97, :])
            else:
                nc.scalar.copy(avs[0:97, 0:512], av[0:97, 0:512])
                nc.vector.tensor_copy(avs[0:97, 512:1024], av[0:97, 512:1024])

            trp = tr_pool.tile([128, 1024], F32)
            for b in range(2):
                for c in range(4):
                    nc.tensor.transpose(
                        trp[:, b * 512 + c * 128:b * 512 + (c + 1) * 128],
                        avs[:, b * 512 + c * 128:b * 512 + (c + 1) * 128],
                        ident[:],
                    )

            rec = rec_pool.tile([128, 16], F32, tag="rec")
            nc.vector.reciprocal(
                rec[:].rearrange("p (x e) -> p x e", e=1),
                trp[:].rearrange("p (x e) -> p x e", e=64)[:, :, 32:33],
            )

            fout = fout_pool.tile([128, 512], F32, tag="fout")
            for b in range(2):
                in0 = trp[:, b * 512:(b + 1) * 512].rearrange(
                    "p (c j e) -> p c j e", j=2, e=64
                )[:, :, :, 0:32]
                outap = fout[:].rearrange(
                    "p (c x d) -> p c x d", x=4, d=32
                )[:, :, b * 2:b * 2 + 2, :]
                recap = rec[:].rearrange(
                    "p (bb c j) -> p bb c j", bb=2, c=4
                )[:, b].unsqueeze(3).broadcast_to((128, 4, 2, 32))
                nc.vector.tensor_mul(outap, in0, recap)

            nc.sync.dma_start(
                out=o_ext[t].rearrange("(c p) d -> p c d", p=128)[
                    :, :, half * 128:(half + 1) * 128
                ],
                in_=fout[:].rearrange("p (c d) -> p c d", d=128),
            )

        def emit_av_out_direct(t, half, exps):
            # exp-stationary AV: out[q, d] directly; rhs = [V_h | 1 | pad]
            # (36 cols); denominator lands at col 32 of each head group
            av2_a = av_pool.tile([128, 2, 4, 36], F32, tag="av2a")
            av2_b = av_pool.tile([128, 2, 4, 36], F32, tag="av2b")
            av2 = [av2_a, av2_b]
            for kc in range(4):
                for c in range(4):
                    for h in range(4):
                        nc.tensor.matmul(
                            av2[c // 2][:, c % 2, h, :],
                            lhsT=exps[kc, h // 2][
                                :, (h % 2) * 512 + c * 128:
                                (h % 2) * 512 + (c + 1) * 128
                            ],
                            rhs=vset[t % 2, half][:, kc, h, :],
                            start=(kc == 0), stop=(kc == 3),
                        )

            rec = rec_pool.tile([128, 2, 2, 4], F32, tag="rec")
            fout = fout_pool.tile([128, 512], fout_dt, tag="fout")
            fv = fout[:].rearrange(
                "p (i cc h d) -> p i cc h d", i=2, cc=2, d=32
            )
            for i in range(2):
                nc.vector.reciprocal(
                    rec[:, i], av2[i][:, :, :, 32:33].rearrange(
                        "p cc h d -> p cc (h d)")
                )
                nc.vector.tensor_mul(
                    fv[:, i],
                    av2[i][:, :, :, 0:32],
                    rec[:, i].unsqueeze(3).broadcast_to((128, 2, 4, 32)),
                )

            out_ap2 = o_ext[t].rearrange("(c p) d -> p c d", p=128)[
                :, :, half * 128:(half + 1) * 128
            ]
            fin2 = fout[:].rearrange("p (c d) -> p c d", d=128)
            if OUT_F32:
                nc.sync.dma_start(out=out_ap2, in_=fin2)
            else:
                nc.gpsimd.dma_start(out=out_ap2, in_=fin2)

        def emit_av_out_skip(t, half, exps):
            # probe: no AV/output stage at all (correctness will fail)
            pass

        emit_av_out = (emit_av_out_skip if (SKIP_AV or QK_ONLY)
                       else emit_av_out_direct if direct
                       else emit_av_out_pe if pe_out
                       else emit_av_out_xbar)

        if not direct and not pe_out:
            # pre-zero the avs ring so xbar reads of rows 97-127 see data
            for _ in range(3):
                avs_z = avs_pool.tile([128, 1024], F16, tag="avs")
                avs_init.append(avs_z)

        qkT = {}
        steps = [t for _ in range(reps) for t in range(T)]
        pending = []
        first = True
        for j in range(len(steps)):
            t = steps[j]
            for half in range(2):
                if half == 0:
                    if first:
                        emit_qk_loads(j, t)
                        emit_preamble()
                        first = False
                    if j + 1 < len(steps):
                        emit_qk_loads(j + 1, steps[j + 1])
                    emit_v_load(t)
                pending.append((t, half, emit_qk_exp(j, t, half)))
                if len(pending) > PIPE_STAGES:
                    emit_av_out(*pending.pop(0))
        for item in pending:
            emit_av_out(*item)


def _attention_body_v3(tc, o_ext, q_ext, k_ext, v_ext, reps=1):
    """Explicitly interleaved 1-half-lag pipeline (OUT_PATH=pe3).

    PSUM: 3-deep score ring (6 banks) whose rotation also serves the AV
    output tile (9th alloc per half), + 1 transpose tile (2 banks) = 8.
    Per half h the emission order keeps every engine's queue stall-free:
      spill(h-1) -> [QK kc0,kc1 + exps](h) -> tr blocks 0-3 (h-1) ->
      [QK kc2 + exps](h) -> tr blocks 4-7 (h-1) -> [QK kc3 + exps](h) ->
      AV matmuls (h) -> recip+mul(h-1) -> out DMA (h-1).
    """
    nc = tc.nc
    Exp = mybir.ActivationFunctionType.Exp

    with (
        tc.tile_pool(name="const", bufs=1) as const_pool,
        tc.tile_pool(name="qk", bufs=3) as qk_pool,
        tc.tile_pool(name="vsb", bufs=4) as v_pool,
        tc.tile_pool(name="exp", bufs=24) as exp_pool,
        tc.tile_pool(name="avs", bufs=3) as avs_pool,
        tc.tile_pool(name="rec", bufs=3) as rec_pool,
        tc.tile_pool(name="fout", bufs=4) as fout_pool,
        tc.tile_pool(name="scorep", bufs=3, space="PSUM") as scores_pool,
        tc.tile_pool(name="trp", bufs=1, space="PSUM") as tr_pool,
    ):
        ident = const_pool.tile([128, 128], F32)
        warm = const_pool.tile([128, 1], F32)
        warm_src = const_pool.tile([128, 1], F32)
        vset = {}
        for s in range(2):
            for half in range(2):
                vsb = v_pool.tile([128, 4, 132], V_DTYPE, tag="vsb")
                vset[s, half] = vsb
        qkT = {}

        def emit_preamble():
            from concourse.masks import make_identity
            nc.gpsimd.memset(warm_src[:], 0.0)
            nc.scalar.activation(warm[:], warm_src[:], Exp)
            make_identity(nc, ident[:])
            for s in range(2):
                for half in range(2):
                    nc.gpsimd.memset(vset[s, half][:], 1.0)

        def emit_qk_loads(j, t):
            qnat = qk_pool.tile([128, 4, 256], QK_DTYPE, tag="qnat")
            knat = qk_pool.tile([128, 4, 256], QK_DTYPE, tag="knat")
            for half in range(2):
                cols = slice(half * 128, (half + 1) * 128)
                nc.gpsimd.dma_start(
                    out=qnat[:, :, cols],
                    in_=q_ext[t].rearrange("(c p) d -> p c d", p=128)[
                        :, :, cols],
                )
                nc.gpsimd.dma_start(
                    out=knat[:, :, cols],
                    in_=k_ext[t].rearrange("(c p) d -> p c d", p=128)[
                        :, :, cols],
                )
            qT = qk_pool.tile([128, 2, 512], QK_DTYPE, tag="qT")
            kT = qk_pool.tile([128, 2, 512], QK_DTYPE, tag="kT")
            for half in range(2):
                for c in range(4):
                    nc.sync.dma_start_transpose(
                        out=qT[:, half, c * 128:(c + 1) * 128],
                        in_=qnat[:, c, half * 128:(half + 1) * 128],
                    )
                    nc.sync.dma_start_transpose(
                        out=kT[:, half, c * 128:(c + 1) * 128],
                        in_=knat[:, c, half * 128:(half + 1) * 128],
                    )
            qkT[j] = (qT, kT)

        def emit_v_load(t):
            vtmp = qk_pool.tile([128, 4, 256], V_DTYPE, tag="vtmp")
            nc.gpsimd.dma_start(
                out=vtmp[:], in_=v_ext[t].rearrange("(kc p) d -> p kc d", p=128)
            )
            eng = nc.gpsimd if VINT_ENGINE == "gpsimd" else nc.vector
            for half in range(2):
                eng.tensor_copy(
                    vset[t % 2, half][:].rearrange(
                        "p kc (h x) -> p kc h x", x=33
                    )[:, :, :, 0:32],
                    vtmp[:, :, half * 128:(half + 1) * 128].rearrange(
                        "p kc (h d) -> p kc h d", d=32
                    ),
                )

        def emit_qk_kc(j, t, half, kc, exps):
            qT, kT = qkT[j]
            for pr in range(2):
                scores = scores_pool.tile([128, 1024], F32, tag="sc")
                _emit_qk_mms(nc, scores, kT, qT, half, kc, pr)
                exp_t = exp_pool.tile([128, 1024], EXP_DTYPE, tag="exp")
                if ACT_PAT[kc * 2 + pr]:
                    nc.scalar.activation(exp_t[:], scores[:], Exp, scale=SCALE)
                else:
                    nc.vector._custom_dve(
                        SCHR_OP,
                        out=exp_t[:].bitcast(mybir.dt.int16),
                        in0=scores[:],
                        s0=SCHR_C0, s1=SCHR_C1,
                    )
                exps[kc, pr] = exp_t

        def emit_av(t, half, exps):
            # av shares the score-ring rotation (9th alloc this half)
            av = scores_pool.tile([128, 1024], F32, tag="sc")
            if AV_KC_INNER:
                order = [(kc, b, jj) for b in range(2) for jj in range(2)
                         for kc in range(4)]
            else:
                order = [(kc, b, jj) for kc in range(4) for b in range(2)
                         for jj in range(2)]
            for kc, b, jj in order:
                i = b * 2 + jj
                nc.tensor.matmul(
                    av[64 * jj:64 * jj + 33, b * 512:(b + 1) * 512],
                    lhsT=vset[t % 2, half][:, kc, i * 33:(i + 1) * 33],
                    rhs=exps[kc, i // 2][
                        :, (i % 2) * 512:(i % 2 + 1) * 512],
                    start=(kc == 0), stop=(kc == 3),
                )
            return av

        def emit_spill(st):
            av, avs = st["av"], None
            avs = avs_pool.tile([128, 1024], F32, tag="avs")
            nc.scalar.copy(avs[0:97, :], av[0:97, :])
            st["avs"] = avs

        def emit_tr(st, blocks):
            avs = st["avs"]
            if "trp" not in st:
                trp_t = tr_pool.tile([128, 1024], F32)
                st["trp"] = trp_t
            trp = st["trp"]
            for x in blocks:
                nc.tensor.transpose(
                    trp[:, x * 128:(x + 1) * 128],
                    avs[:, x * 128:(x + 1) * 128],
                    ident[:],
                )

        def emit_out(st):
            t, half, trp = st["t"], st["half"], st["trp"]
            rec = rec_pool.tile([128, 16], F32, tag="rec")
            nc.vector.reciprocal(
                rec[:].rearrange("p (x e) -> p x e", e=1),
                trp[:].rearrange("p (x e) -> p x e", e=64)[:, :, 32:33],
            )
            fout = fout_pool.tile([128, 512], F32, tag="fout")
            for b in range(2):
                in0 = trp[:, b * 512:(b + 1) * 512].rearrange(
                    "p (c j e) -> p c j e", j=2, e=64
                )[:, :, :, 0:32]
                outap = fout[:].rearrange(
                    "p (c x d) -> p c x d", x=4, d=32
                )[:, :, b * 2:b * 2 + 2, :]
                recap = rec[:].rearrange(
                    "p (bb c j) -> p bb c j", bb=2, c=4
                )[:, b].unsqueeze(3).broadcast_to((128, 4, 2, 32))
                nc.vector.tensor_mul(outap, in0, recap)
            nc.sync.dma_start(
                out=o_ext[t].rearrange("(c p) d -> p c d", p=128)[
                    :, :, half * 128:(half + 1) * 128],
                in_=fout[:].rearrange("p (c d) -> p c d", d=128),
            )

        steps = [t for _ in range(reps) for t in range(T)]
        prev = None
        first = True
        for j in range(len(steps)):
            t = steps[j]
            for half in range(2):
                if half == 0:
                    if first:
                        emit_qk_loads(j, t)
                        emit_preamble()
                        first = False
                    if j + 1 < len(steps):
                        emit_qk_loads(j + 1, steps[j + 1])
                    emit_v_load(t)
                if prev is not None:
                    emit_spill(prev)
                exps = {}
                emit_qk_kc(j, t, half, 0, exps)
                emit_qk_kc(j, t, half, 1, exps)
                if prev is not None:
                    emit_tr(prev, range(0, 4))
                emit_qk_kc(j, t, half, 2, exps)
                if prev is not None:
                    emit_tr(prev, range(4, 8))
                emit_qk_kc(j, t, half, 3, exps)
                av = emit_av(t, half, exps)
                if prev is not None:
                    emit_out(prev)
                prev = {"t": t, "half": half, "av": av}
        emit_spill(prev)
        emit_tr(prev, range(0, 8))
        emit_out(prev)


def build_program(enable_asserts=False, reps=1):
    nc = bacc.Bacc(
        "TRN2",
        target_bir_lowering=False,
        debug=False,
        enable_asserts=enable_asserts,
        num_devices=NCORES,
    )
    q_ext = nc.dram_tensor("q", [T, N, D], F32, kind="ExternalInput").ap()
    k_ext = nc.dram_tensor("k", [T, N, D], F32, kind="ExternalInput").ap()
    v_ext = nc.dram_tensor("v", [T, N, D], F32, kind="ExternalInput").ap()
    o_ext = nc.dram_tensor("out", [T, N, D], F32, kind="ExternalOutput").ap()
    with tile.TileContext(nc) as tc:
        if OUT_PATH == "pe3" and not (SKIP_AV or QK_ONLY):
            _attention_body_v3(tc, o_ext, q_ext, k_ext, v_ext, reps=reps)
        else:
            _attention_body(tc, o_ext, q_ext, k_ext, v_ext, reps=reps)
    nc.compile()
    return nc


_NC_CACHE = None


def _get_nc():
    global _NC_CACHE
    if _NC_CACHE is None:
        _NC_CACHE = build_program()
    return _NC_CACHE


def run(query, key, value, trace=False):
    """Run on the 8 NeuronCores; returns (out, exec_time_ns_or_None)."""
    nc = _get_nc()
    in_maps = [
        {
            "q": np.ascontiguousarray(np.asarray(query[i], dtype=np.float32)),
            "k": np.ascontiguousarray(np.asarray(key[i], dtype=np.float32)),
            "v": np.ascontiguousarray(np.asarray(value[i], dtype=np.float32)),
        }
        for i in range(NCORES)
    ]
    res = run_bass_kernel_spmd(nc, in_maps, core_ids=list(range(NCORES)),
                               trace=trace)
    out = np.stack([np.asarray(res.results[i]["out"]) for i in range(NCORES)])
    return out, res.exec_time_ns


def kernel(query, key, value):
    out, _ = run(query, key, value, trace=False)
    return out
